# revision 2
# baseline (speedup 1.0000x reference)
"""Trainium2 Bass kernel for nn_Encoder_Postnet_combine (B=16,T=4096,P=512,D=512,S=100).

Math (algebraically folded from the reference):
  idx[b,t]   : sequential aligner scan (host, tiny integer recurrence)
  W1 = w_out[:D]; W2 = w_out[D:]
  Wc  = (I + w_pos) @ W1
  EW  = encoder_out @ Wc                       (device GEMM, per batch)
  v   = w_pitch[0] @ W1
  dEb = (emb_beats[1]-emb_beats[0]) @ W1
  EsW = emb_singer @ W2
  PEW = pe @ (w_pos @ W1) + (b_pitch+b_pos+emb_beats[0]) @ W1 + b_out
  out = leaky( EW[b,idx] + EsW[sv] + PEW[t] + pitch*v + beats*dEb , 0.01)

Sharding: data-parallel over batch, 2 batches per core on 8 cores.
"""
import numpy as np

import concourse.bass as bass
import concourse.mybir as mybir
import concourse.tile as tile
from concourse.vector_clock import ScopedClock
from concourse.bass_utils import run_bass_kernel_spmd

F32 = mybir.dt.float32
F32R = mybir.dt.float32r
I32 = mybir.dt.int32

B, T, PH, D, S = 16, 4096, 512, 512, 100
NCORES = 8
BPC = B // NCORES          # batches per core
TT = T // 128              # 32 t-tiles per batch
NT = BPC * TT              # 64 tiles per core

# ---------------------------------------------------------------------------
# Workarounds for this walrus build: at most ONE sync wait per instruction
# (EventSemaphore: 2).


def _split_drain_and_barrier(self, tick_clock, wait_clock):
    nc = self.nc
    probe = nc.sync.nop()
    wait_clock.add_sem_waits(probe.ins, ScopedClock({None: tick_clock.global_clock}))
    si = probe.ins.sync_info
    if si is not None and si.on_wait and len(si.on_wait) > 1:
        waits = list(si.on_wait)
        si.on_wait = waits[:1]
        for w in waits[1:]:
            extra = nc.sync.nop()
            extra.ins.sync_info = mybir.SyncInfo(on_wait=[w], on_update=[])
    nc.sync.drain()
    nc.all_engine_barrier()
    assert self.sems is not None
    popped = nc._tile_sem_poison_stack.pop()
    assert popped is self._sem_poison
    nc.clear_and_free_semaphores(list(self.sems.allocated().values()))
    nc.all_engine_barrier()


tile.TileContext._drain_and_barrier = _split_drain_and_barrier


def _split_multi_waits(nc):
    counter = [0]

    def fresh_nop(engine, wait):
        counter[0] += 1
        nop = mybir.InstNoOp(name=f"waitsplit_{counter[0]}", ins=[], outs=[])
        nop.engine = engine
        nop.sync_info = mybir.SyncInfo(on_wait=[wait], on_update=[])
        return nop

    for fn in nc.m.functions:
        for blk in fn.blocks:
            new_insts = []
            for inst in blk.instructions:
                si = inst.sync_info
                limit = 2 if isinstance(inst, mybir.InstEventSemaphore) else 1
                if si is not None and si.on_wait and len(si.on_wait) > limit:
                    waits = list(si.on_wait)
                    for w in waits[:-limit]:
                        new_insts.append(fresh_nop(inst.engine, w))
                    si.on_wait = waits[-limit:]
                new_insts.append(inst)
            blk.instructions = new_insts


# ---------------------------------------------------------------------------
# Device program


def build_program(repeat=1):
    nc = bass.Bass()
    encT = nc.declare_dram_parameter("encT", [BPC * PH, D], F32R, isOutput=False)
    wc = nc.declare_dram_parameter("wc", [D, D], F32R, isOutput=False)
    pew = nc.declare_dram_parameter("pew", [T, D], F32, isOutput=False)
    esw = nc.declare_dram_parameter("esw", [128, D], F32, isOutput=False)
    gidx = nc.declare_dram_parameter("gidx", [128, NT], I32, isOutput=False)
    sidx = nc.declare_dram_parameter("sidx", [128, NT], I32, isOutput=False)
    pcol = nc.declare_dram_parameter("pcol", [128, NT], F32, isOutput=False)
    bcol = nc.declare_dram_parameter("bcol", [128, NT], F32, isOutput=False)
    vrep = nc.declare_dram_parameter("vrep", [128, 2 * D], F32, isOutput=False)
    out = nc.declare_dram_parameter("out", [BPC * T, D], F32, isOutput=True)
    ew_dram = nc.dram_tensor("ew_dram", [BPC * PH, D], F32)

    with tile.TileContext(nc) as tc:
        with (
            tc.tile_pool(name="const", bufs=1) as cpool,
            tc.tile_pool(name="sbuf", bufs=4) as pool,
            tc.tile_pool(name="psum", bufs=4, space="PSUM") as psum,
        ):
            def body(_=None):
                # --- constants / small inputs ---
                vdeb = cpool.tile([128, 2 * D], F32, tag="vdeb")
                nc.sync.dma_start(out=vdeb[:], in_=vrep[:])
                gidx_sb = cpool.tile([128, NT], I32, tag="gidx")
                nc.sync.dma_start(out=gidx_sb[:], in_=gidx[:])
                sidx_sb = cpool.tile([128, NT], I32, tag="sidx")
                nc.sync.dma_start(out=sidx_sb[:], in_=sidx[:])
                pcol_sb = cpool.tile([128, NT], F32, tag="pcol")
                nc.sync.dma_start(out=pcol_sb[:], in_=pcol[:])
                bcol_sb = cpool.tile([128, NT], F32, tag="bcol")
                nc.sync.dma_start(out=bcol_sb[:], in_=bcol[:])

                # --- phase A: EW = E @ Wc (per batch), bounce to DRAM ---
                wc_sb = []
                for ki in range(4):
                    w_t = cpool.tile([128, D], F32R, tag=f"wc{ki}")
                    nc.sync.dma_start(out=w_t[:], in_=wc[ki * 128:(ki + 1) * 128, :])
                    wc_sb.append(w_t)
                encT_sb = []
                for j in range(4 * BPC):
                    e_t = cpool.tile([128, D], F32R, tag=f"encT{j}")
                    nc.sync.dma_start(out=e_t[:], in_=encT[j * 128:(j + 1) * 128, :])
                    encT_sb.append(e_t)
                for b in range(BPC):
                    for mm in range(4):
                        ps = psum.tile([128, D], F32, tag="ps_ew")
                        for ki in range(4):
                            nc.tensor.matmul(
                                out=ps[:],
                                lhsT=encT_sb[b * 4 + ki][:, mm * 128:(mm + 1) * 128],
                                rhs=wc_sb[ki][:],
                                start=(ki == 0),
                                stop=(ki == 3),
                            )
                        ew_t = pool.tile([128, D], F32, tag="ew_t")
                        nc.vector.tensor_copy(out=ew_t[:], in_=ps[:])
                        r0 = b * PH + mm * 128
                        nc.sync.dma_start(out=ew_dram[r0:r0 + 128, :], in_=ew_t[:])

                # --- phase B: gather + fused elementwise + leaky ---
                for tt in range(TT):
                    pew_t = pool.tile([128, D], F32, tag="pew_t")
                    nc.sync.dma_start(out=pew_t[:], in_=pew[tt * 128:(tt + 1) * 128, :])
                    for b in range(BPC):
                        k = tt * BPC + b
                        g1 = pool.tile([128, D], F32, tag="g1")
                        nc.gpsimd.indirect_dma_start(
                            out=g1[:], out_offset=None, in_=ew_dram[:],
                            in_offset=bass.IndirectOffsetOnAxis(ap=gidx_sb[:, k:k + 1], axis=0))
                        g2 = pool.tile([128, D], F32, tag="g2")
                        nc.gpsimd.indirect_dma_start(
                            out=g2[:], out_offset=None, in_=esw[:],
                            in_offset=bass.IndirectOffsetOnAxis(ap=sidx_sb[:, k:k + 1], axis=0))
                        s1 = pool.tile([128, D], F32, tag="s1")
                        nc.vector.tensor_tensor(out=s1[:], in0=g1[:], in1=g2[:],
                                                op=mybir.AluOpType.add)
                        s2 = pool.tile([128, D], F32, tag="s2")
                        nc.vector.scalar_tensor_tensor(
                            out=s2[:], in0=vdeb[:, :D], scalar=pcol_sb[:, k:k + 1],
                            in1=s1[:], op0=mybir.AluOpType.mult, op1=mybir.AluOpType.add)
                        s3 = pool.tile([128, D], F32, tag="s3")
                        nc.vector.scalar_tensor_tensor(
                            out=s3[:], in0=vdeb[:, D:], scalar=bcol_sb[:, k:k + 1],
                            in1=s2[:], op0=mybir.AluOpType.mult, op1=mybir.AluOpType.add)
                        s4 = pool.tile([128, D], F32, tag="s4")
                        nc.vector.tensor_tensor(out=s4[:], in0=s3[:], in1=pew_t[:],
                                                op=mybir.AluOpType.add)
                        o_t = pool.tile([128, D], F32, tag="o_t")
                        nc.scalar.activation(out=o_t[:], in_=s4[:],
                                             func=mybir.ActivationFunctionType.Lrelu,
                                             alpha=0.01)
                        r0 = b * T + tt * 128
                        nc.sync.dma_start(out=out[r0:r0 + 128, :], in_=o_t[:])

            if repeat == 1:
                body()
            else:
                with tc.For_i(0, repeat, 1) as _i:
                    body()

    _split_multi_waits(nc)
    return nc


# ---------------------------------------------------------------------------
# Host side


def _host_scan_idx(align, text):
    align = np.asarray(align, dtype=np.int64)
    text = np.asarray(text, dtype=np.int64)
    Bn, Tn = align.shape
    Pn = text.shape[1]
    idx = np.zeros((Bn, Tn), dtype=np.int32)
    ind = np.zeros(Bn, dtype=np.int64)
    rows = np.arange(Bn)
    cur = text[rows, ind]
    for t in range(1, Tn):
        a = align[:, t]
        stay = a == cur
        ind = np.where(stay, ind, np.minimum(ind + 1, Pn - 1))
        cur = np.where(stay, cur, text[rows, ind])
        idx[:, t] = ind
    return idx


def _positional_encoding(length, d_model):
    pos = np.arange(length, dtype=np.float32)[:, None]
    div = np.exp(np.arange(0, d_model, 2, dtype=np.float32)
                 * (-np.log(10000.0) / d_model))
    pe = np.zeros((length, d_model), np.float32)
    pe[:, 0::2] = np.sin(pos * div)
    pe[:, 1::2] = np.cos(pos * div)
    return pe


def _fold(w_pitch, b_pitch, w_pos, b_pos, emb_beats, emb_singer, w_out, b_out):
    f64 = np.float64
    W1 = np.asarray(w_out[:D], f64)
    W2 = np.asarray(w_out[D:], f64)
    WposW1 = np.asarray(w_pos, f64) @ W1
    Wc = (W1 + WposW1).astype(np.float32)
    v = (np.asarray(w_pitch[0], f64) @ W1).astype(np.float32)
    EbW = np.asarray(emb_beats, f64) @ W1
    dEb = (EbW[1] - EbW[0]).astype(np.float32)
    EsW = (np.asarray(emb_singer, f64) @ W2).astype(np.float32)
    cb = (np.asarray(b_pitch + b_pos, f64) @ W1 + EbW[0] + np.asarray(b_out, f64))
    pe = _positional_encoding(T, D)
    PEW = (np.asarray(pe, f64) @ WposW1 + cb[None, :]).astype(np.float32)
    return Wc, v, dEb, EsW, PEW


def _tile_cols(x_core):
    """[BPC, T] -> [128, NT] where col (tt*BPC+b)[p] = x[b, tt*128+p]."""
    a = x_core.reshape(BPC, TT, 128)          # [b, tt, p]
    a = np.transpose(a, (2, 1, 0))            # [p, tt, b]
    return np.ascontiguousarray(a.reshape(128, NT))


_CACHE = {}


def kernel(encoder_out, align_phone, text_phone, pitch, beats, singer_vec,
           w_pitch, b_pitch, w_pos, b_pos, emb_beats, emb_singer, w_out, b_out):
    encoder_out = np.ascontiguousarray(np.asarray(encoder_out, np.float32))
    pitch = np.asarray(pitch, np.float32)[..., 0]          # [B,T]
    beats_f = np.asarray(beats, np.int64)[..., 0].astype(np.float32)
    sv = np.asarray(singer_vec, np.int64)[..., 0].astype(np.int32)  # [B,T]

    idx = _host_scan_idx(align_phone, text_phone)          # [B,T] int32
    Wc, v, dEb, EsW, PEW = _fold(
        np.asarray(w_pitch, np.float32), np.asarray(b_pitch, np.float32),
        np.asarray(w_pos, np.float32), np.asarray(b_pos, np.float32),
        np.asarray(emb_beats, np.float32), np.asarray(emb_singer, np.float32),
        np.asarray(w_out, np.float32), np.asarray(b_out, np.float32))

    esw_pad = np.zeros((128, D), np.float32)
    esw_pad[:S] = EsW
    vrep = np.ascontiguousarray(
        np.broadcast_to(np.concatenate([v, dEb])[None, :], (128, 2 * D)))

    if "nc" not in _CACHE:
        _CACHE["nc"] = build_program()
    nc = _CACHE["nc"]

    in_maps = []
    for c in range(NCORES):
        b0 = c * BPC
        sl = slice(b0, b0 + BPC)
        encT = np.ascontiguousarray(
            encoder_out[sl].transpose(0, 2, 1).reshape(BPC * PH, D))
        idx_c = idx[sl]                                    # [BPC, T]
        gidx = _tile_cols(idx_c + (np.arange(BPC, dtype=np.int32)[:, None] * PH))
        in_maps.append({
            "encT": encT,
            "wc": Wc,
            "pew": PEW,
            "esw": esw_pad,
            "gidx": gidx.astype(np.int32),
            "sidx": _tile_cols(sv[sl]).astype(np.int32),
            "pcol": _tile_cols(pitch[sl]).astype(np.float32),
            "bcol": _tile_cols(beats_f[sl]).astype(np.float32),
            "vrep": vrep,
        })

    _CACHE["last_in_maps"] = in_maps
    res = run_bass_kernel_spmd(nc, in_maps, core_ids=list(range(NCORES)))
    out = np.empty((B, T, D), np.float32)
    for c in range(NCORES):
        out[c * BPC:(c + 1) * BPC] = res.results[c]["out"].reshape(BPC, T, D)
    return out
